# revision 35
# baseline (speedup 1.0000x reference)
"""Trainium2 Bass kernel for MultiHeadAttention with relative position bias.

Reference computation (B=2, S=2048, D=1024, H=16, Dk=64, MAX_REL=128):
    Q,K,V = x@W{q,k,v}.T + b      (per-head reshape)
    scores = QK^T/sqrt(Dk) + rel_bias_matrix
    out = softmax(scores) @ V, heads merged, @ Wo.T + bo

Sharding (8 cores): core c handles batch b=c//4 and 4 heads hg=4*(c%4)..+4
(data + head parallel). Q/K/V projections column-split per head group,
Wo row-split; the partial outputs are summed on the host (the "all-reduce").

Per-core device algorithm (channels-on-partitions transposed layouts, bf16
operands everywhere with f32 PSUM accumulation):

The schedule is built around the exp stream on the Activation engine (the
long pole: 128 x ~1.15us per [128,1024] score chunk).  PSUM is the scarce
resource: the double-buffered score tiles take 4 banks and one unit's PV
accumulators take the other 4, so no other PSUM-using work can coexist
with an accumulating unit.  Hence a chain-deferred PV schedule:

  pre:  Q/K projection flights for both head pairs (ACT idle anyway).
  kc stream (64 steps, one per (unit, k-chunk)): QK matmuls -> exp ->
        band/future fixups -> P~ tile (18-deep pool).  The PV matmuls of
        unit u run DELAY=16 steps later, i.e. during unit u+1's stream,
        using the 4 PSUM banks unit u+1's own PV is *not* using.  The V
        projection weaves into unit 0's stream (whose inflow PV is empty).
  drain: unit 3's PV + softmax-normalize interleaved with the first half
        of the Wo chunks, then the rest of Wo; bf16 partial out via DMA.

P~ = exp(s/8 - 2) * band/future multipliers is the true softmax numerator
up to a constant per-head factor which cancels in the normalization.  The
denominator comes free via a ones-column on V; it is DMA'd from PSUM to
DRAM, broadcast-read back, inverted with the fast approximate reciprocal,
and multiplied into C^T.
"""

import math
import os
import sys

for _p in ("/opt/trn_rl_repo", "/root/.axon_site", "/root/.axon_site/_ro/trn_rl_repo",
           "/root/.axon_site/_ro/pypackages"):
    if os.path.isdir(_p) and _p not in sys.path:
        sys.path.append(_p)

import numpy as np
import ml_dtypes

import concourse.bass as bass
import concourse.mybir as mybir
import concourse.tile as tile
from concourse import bacc, library_config
from contextlib import ExitStack

# Problem constants (hardcoded per the contract).
B, S, D = 2, 2048, 1024
H, DK = 16, 64
MAX_REL = 128
N_CORES = 8
CORES_PER_BATCH = 4
HEADS_PER_CORE = H // CORES_PER_BATCH  # 4
CL = HEADS_PER_CORE * DK               # 256 local channels
N_PAIRS = HEADS_PER_CORE // 2          # 2 head pairs
QH = 1024                              # q processed in halves
N_QH = S // QH                         # 2
N_KC = S // 128                        # 16 k chunks
BAND = 3 * 128                         # band width in q for one k chunk

F32 = mybir.dt.float32
BF16 = mybir.dt.bfloat16

SCALE = 1.0 / math.sqrt(DK)
# Constant bias in the exp keeps P~ in a comfortable range; the e^EXP_BIAS
# factor scales numerator and denominator alike and cancels.
EXP_BIAS = -2.0

EXP = mybir.ActivationFunctionType.Exp

UNITS = ((0, 0), (0, 1), (1, 0), (1, 1))   # (pair, qh) stream order
DELAY = N_KC                               # PV trails the exp stream by 1 unit


def build_program(reps=1):
    nc = bacc.Bacc("TRN2", target_bir_lowering=False, debug=False)

    xt_d = nc.declare_dram_parameter("xt", [D, S], BF16, isOutput=False)
    wqt_d = nc.declare_dram_parameter("wqt", [D, CL], BF16, isOutput=False)
    wkt_d = nc.declare_dram_parameter("wkt", [D, CL], BF16, isOutput=False)
    wvt_d = nc.declare_dram_parameter("wvt", [D, CL], BF16, isOutput=False)
    wot_d = nc.declare_dram_parameter("wot", [CL, D], BF16, isOutput=False)
    bqk_d = nc.declare_dram_parameter("bqk", [128, 4], F32, isOutput=False)
    band_d = nc.declare_dram_parameter("band", [128, HEADS_PER_CORE, BAND], BF16,
                                       isOutput=False)
    # per-head constants replicated over partitions: [:, 2h] = future
    # multiplier exp(c_fut - c_past)
    abias_d = nc.declare_dram_parameter("abias", [128, 2 * HEADS_PER_CORE], F32,
                                        isOutput=False)
    out_d = nc.declare_dram_parameter("out_p", [S, D], BF16, isOutput=True)
    # scratch for the denominator broadcast: [pair*4 + hh*2 + qh, q]
    den_d = nc.dram_tensor("den_scratch", [1, 2 * N_PAIRS * N_QH, QH], F32)

    with tile.TileContext(nc) as tc, ExitStack() as ctx:
        # ---------- long-lived SBUF ----------
        persist = ctx.enter_context(tc.tile_pool(name="persist", bufs=1))
        q_sb = persist.tile([128, 2, S], BF16, tag="q_sb")
        k_sb = persist.tile([128, 2, S], BF16, tag="k_sb")
        v_sb = persist.tile([128, N_KC, HEADS_PER_CORE, DK + 1], BF16, tag="v_sb")
        ct_sb = persist.tile([128, 2, S], BF16, tag="ct_sb")
        wo_sb = persist.tile([128, 2, D], BF16, tag="wo_sb")
        band_sb = persist.tile([128, HEADS_PER_CORE, BAND], BF16, tag="band_sb")
        bqk_sb = persist.tile([128, 4], F32, tag="bqk_sb")
        abias_sb = persist.tile([128, 2 * HEADS_PER_CORE], F32, tag="abias_sb")
        ebias_sb = persist.tile([128, 1], F32, tag="ebias_sb")
        nc.vector.memset(ebias_sb, EXP_BIAS)

        nc.gpsimd.load_library(library_config.attn)
        nc.sync.dma_start(out=bqk_sb, in_=bqk_d.ap())
        nc.sync.dma_start(out=abias_sb, in_=abias_d.ap())
        nc.sync.dma_start(out=band_sb, in_=band_d.ap())

        # ---------- PSUM pools: stp = exp-feeding score tiles (4 banks),
        # accp = projection/PV/Wo accumulators (4 banks) ----------
        stp = ctx.enter_context(tc.tile_pool(name="stp", bufs=2, space="PSUM"))
        accp = ctx.enter_context(tc.tile_pool(name="accp", bufs=2, space="PSUM"))
        outp = ctx.enter_context(tc.tile_pool(name="outp", bufs=6))

        sb = dict(q=q_sb, k=k_sb, v=v_sb, ct=ct_sb, wo=wo_sb, band=band_sb,
                  bqk=bqk_sb, abias=abias_sb, ebias=ebias_sb)
        dram = dict(xt=xt_d, wqt=wqt_d, wkt=wkt_d, wvt=wvt_d, wot=wot_d,
                    out=out_d, den=den_d)
        pools = dict(stp=stp, accp=accp, outp=outp)

        for rep in range(reps):
            _phases(nc, tc, sb, dram, pools, rep)

    nc.compile()
    return nc


_DONE = object()


def _consume(gen, n):
    for _ in range(n):
        if next(gen, _DONE) is _DONE:
            break


def _phases(nc, tc, sb, dram, pools, rep):
    q_sb, k_sb, v_sb, ct_sb, wo_sb = sb["q"], sb["k"], sb["v"], sb["ct"], sb["wo"]
    band_sb, bqk_sb, abias_sb, ebias_sb = (sb["band"], sb["bqk"], sb["abias"],
                                           sb["ebias"])
    stp, accp, outp = (pools[n] for n in ("stp", "accp", "outp"))

    phase_ctx = ExitStack()
    xw = phase_ctx.enter_context(tc.tile_pool(name=f"xw{rep}", bufs=1))
    xt_sb = xw.tile([128, D // 128, S], BF16, tag="xt_sb")
    wq_sb = xw.tile([128, D // 128, CL], BF16, tag="wq_sb")
    wk_sb = xw.tile([128, D // 128, CL], BF16, tag="wk_sb")
    wv_sb = xw.tile([128, D // 128, CL], BF16, tag="wv_sb")
    # P~ tiles: live from exp until the PV sweep one unit later
    ptp = phase_ctx.enter_context(tc.tile_pool(name=f"ptp{rep}", bufs=DELAY + 2))
    nrm = phase_ctx.enter_context(tc.tile_pool(name=f"nrm{rep}", bufs=2))

    xt_v = dram["xt"].ap().rearrange("(c p) s -> p c s", p=128)
    NDC = D // 128
    GROUPS = (range(0, NDC // 2), range(NDC // 2, NDC))

    # ones column on V gives the softmax denominator for free in the PV matmul
    nc.vector.memset(v_sb[:, :, :, DK:DK + 1], 1.0)

    # Startup DMA order: wq + xt chunks 0-3 unblock the first flight group.
    nc.sync.dma_start(out=wq_sb, in_=dram["wqt"].ap().rearrange("(c p) m -> p c m", p=128))
    for dc in GROUPS[0]:
        nc.sync.dma_start(out=xt_sb[:, dc, :], in_=xt_v[:, dc, :])
    nc.sync.dma_start(out=wk_sb, in_=dram["wkt"].ap().rearrange("(c p) m -> p c m", p=128))
    for dc in GROUPS[1]:
        nc.sync.dma_start(out=xt_sb[:, dc, :], in_=xt_v[:, dc, :])
    nc.sync.dma_start(out=wv_sb, in_=dram["wvt"].ap().rearrange("(c p) m -> p c m", p=128))
    nc.sync.dma_start(out=wo_sb, in_=dram["wot"].ap().rearrange("(c p) m -> p c m", p=128))

    # ---------- emitters ----------
    def qk_flight_t(w_sb, o_sb, boff, j, t, pool, tag, act_evict):
        """One q-half (t) of the Q or K projection for head pair j.  Yields
        after each dc-group of 4 matmuls so it can weave into the exp stream;
        evicts via ACT pre-stream or DVE once ACT is busy with exps."""
        slot = pool.tile([128, 1024], F32, tag=tag, name="pjt")
        for g in GROUPS:
            for half in range(2):
                for dc in g:
                    nc.tensor.matmul(
                        slot[:, half * 512:(half + 1) * 512],
                        lhsT=w_sb[:, dc, j * 128:(j + 1) * 128],
                        rhs=xt_sb[:, dc, t * 1024 + half * 512:
                                  t * 1024 + (half + 1) * 512],
                        start=(dc == 0), stop=(dc == NDC - 1),
                    )
                yield
        if act_evict:
            nc.scalar.add(
                out=o_sb[:, j, t * 1024:(t + 1) * 1024],
                in_=slot,
                add=bqk_sb[:, boff + j:boff + j + 1],
            )
        else:
            nc.vector.tensor_scalar_add(
                out=o_sb[:, j, t * 1024:(t + 1) * 1024],
                in0=slot,
                scalar1=bqk_sb[:, boff + j:boff + j + 1],
            )
        yield

    def v_proj():
        """V projection: [s_chunk, dv] tiles, 4 s-chunks per 2 PSUM slots.
        Yields after each 4-matmul dc-group; evictions on DVE."""
        for scg in range(N_KC // 4):
            ps = accp.tile([128, 1024], F32, tag="acc", name="vps")
            psb = accp.tile([128, 1024], F32, tag="acc", name="vpsb")
            both = (ps, psb)
            for g in GROUPS:
                for i in range(4):
                    sc = scg * 4 + i
                    tgt = both[i // 2]
                    col = (i % 2) * 512
                    for dc in g:
                        nc.tensor.matmul(
                            tgt[:, col:col + CL],
                            lhsT=xt_sb[:, dc, sc * 128:(sc + 1) * 128],
                            rhs=wv_sb[:, dc, :],
                            start=(dc == 0), stop=(dc == NDC - 1),
                        )
                    yield
            for i in range(4):
                sc = scg * 4 + i
                tgt = both[i // 2]
                col = (i % 2) * 512
                nc.vector.tensor_copy(
                    out=v_sb[:, sc, :, 0:DK],
                    in_=tgt[:, col:col + CL].rearrange("p (h d) -> p h d",
                                                       h=HEADS_PER_CORE),
                )
            yield

    def qk_exp(pair, kc, w0, pt2):
        """QK matmuls + exp + band/future fixups for both heads of a pair at
        one k chunk.  pt2: [128, 2, QH] P~ destination."""
        k0 = kc * 128
        for hh in range(2):
            p0 = hh * 64
            st = stp.tile([128, QH], F32, tag="st", name=f"st{hh}")
            for half in range(QH // 512):
                nc.tensor.matmul(
                    st[:, half * 512:(half + 1) * 512],
                    lhsT=k_sb[p0:p0 + 64, pair, k0:k0 + 128],
                    rhs=q_sb[p0:p0 + 64, pair,
                             w0 + half * 512:w0 + (half + 1) * 512],
                    start=True, stop=True,
                    tile_position=(p0, 0),
                )
            nc.scalar.activation(out=pt2[:, hh, :], in_=st, func=EXP,
                                 scale=SCALE, bias=ebias_sb)
        # 2D per-head DVE ops only: 3D APs fall off the DVE fast path.
        fut_end = min(max(k0 - 128, w0), w0 + QH)
        n_fut = fut_end - w0
        b_lo = max(k0 - 128, w0)
        b_hi = min(k0 + 2 * 128, w0 + QH)
        m0 = b_lo - (k0 - 128)
        for hh in range(2):
            h = 2 * pair + hh
            if n_fut > 0:
                nc.vector.tensor_scalar_mul(
                    out=pt2[:, hh, 0:n_fut], in0=pt2[:, hh, 0:n_fut],
                    scalar1=abias_sb[:, 2 * h:2 * h + 1],
                )
            if b_hi > b_lo:
                nc.vector.tensor_mul(
                    out=pt2[:, hh, b_lo - w0:b_hi - w0],
                    in0=pt2[:, hh, b_lo - w0:b_hi - w0],
                    in1=band_sb[:, h, m0:m0 + (b_hi - b_lo)],
                )

    def pv(pair, accs, pt2, kc):
        for hh in range(2):
            for sub in range(QH // 512):
                nc.tensor.matmul(
                    accs[hh][:, sub * 512:(sub + 1) * 512],
                    lhsT=v_sb[:, kc, 2 * pair + hh, :],
                    rhs=pt2[:, hh, sub * 512:(sub + 1) * 512],
                    start=(kc == 0), stop=(kc == N_KC - 1),
                )

    def normalize(pair, qh, accs):
        """Evict C^T + denominators, broadcast, approx-recip, divide."""
        w0 = qh * QH
        den_v = dram["den"].ap()
        s0 = pair * 4 + qh
        den_t = nrm.tile([1, 2, QH], F32, tag="den")
        for hh in range(2):
            nc.vector.tensor_copy(
                out=ct_sb[hh * 64:hh * 64 + 64, pair, w0:w0 + QH],
                in_=accs[hh][0:DK, :])
            nc.vector.tensor_copy(out=den_t[:, hh, :], in_=accs[hh][DK:DK + 1, :])
            sdst = bass.AP(
                tensor=den_v.tensor, offset=den_v.offset + (s0 + 2 * hh) * QH,
                ap=[[1, QH]],
            )
            # gpsimd DGE queue: keeps the normalize chain off the sync
            # queue, which the Wo output flush saturates in the drain
            nc.gpsimd.dma_start(out=sdst, in_=den_t[:, hh, :])
        rbc = nrm.tile([128, QH], F32, tag="rbc")
        for hh in range(2):
            bsrc = bass.AP(
                tensor=den_v.tensor,
                offset=den_v.offset + (s0 + 2 * hh) * QH,
                ap=[[0, 64], [1, QH]],
            )
            nc.gpsimd.dma_start(out=rbc[hh * 64:hh * 64 + 64, :], in_=bsrc)
        nc.vector.reciprocal_approx_fast(out=rbc, in_=rbc)
        # two half-width divides: subtile deps let Wo chunks of the first
        # q-half start without waiting for the full row in the drain
        for half in range(2):
            nc.vector.tensor_mul(
                out=ct_sb[:, pair, w0 + half * 512:w0 + (half + 1) * 512],
                in0=ct_sb[:, pair, w0 + half * 512:w0 + (half + 1) * 512],
                in1=rbc[:, half * 512:(half + 1) * 512],
            )

    def wo_chunk(st_i, ps):
        o_sb = outp.tile([128, D], BF16, tag="o_sb")
        for mt in range(2):
            for j in range(2):
                nc.tensor.matmul(
                    ps[:, mt * 512:(mt + 1) * 512],
                    lhsT=ct_sb[:, j, st_i * 128:(st_i + 1) * 128],
                    rhs=wo_sb[:, j, mt * 512:(mt + 1) * 512],
                    start=(j == 0), stop=(j == 1),
                )
        # alternate eviction engine so neither ACT nor DVE serializes the
        # tail, and alternate DMA queues so the 4MB flush runs at 2x
        if st_i % 2 == 0:
            nc.scalar.copy(out=o_sb, in_=ps)
            eng = nc.sync
        else:
            nc.vector.tensor_copy(out=o_sb, in_=ps)
            eng = nc.scalar
        eng.dma_start(out=dram["out"].ap()[st_i * 128:(st_i + 1) * 128, :],
                      in_=o_sb)

    # ---------- pre-stream: the projection flights the stream needs first.
    # Unit 0 only reads q pair0/half0 and k pair0; pair-1 flights must also
    # finish pre-stream (no PSUM room once PV accumulators go live), but the
    # pair-0 second halves weave into unit 0's stream. ----------
    _consume(qk_flight_t(wq_sb, q_sb, 0, 0, 0, stp, "st", True), 999)
    _consume(qk_flight_t(wk_sb, k_sb, 2, 0, 0, stp, "st", True), 999)
    for t in range(2):
        _consume(qk_flight_t(wq_sb, q_sb, 0, 1, t, accp, "acc", True), 999)
    for t in range(2):
        _consume(qk_flight_t(wk_sb, k_sb, 2, 1, t, accp, "acc", True), 999)

    # ---------- kc stream with chain-deferred PV ----------
    def weave_gen():
        yield from qk_flight_t(wk_sb, k_sb, 2, 0, 1, accp, "acc", False)
        yield from qk_flight_t(wq_sb, q_sb, 0, 0, 1, accp, "acc", False)
        yield from v_proj()

    vgen = weave_gen()
    pts = {}
    accs_by_unit = {}

    def emit_pv(gk):
        u, kc = divmod(gk, N_KC)
        pair, qh = UNITS[u]
        if kc == 0:
            accs_by_unit[u] = [
                accp.tile([DK + 1, QH], F32, tag="acc", name=f"acc{u}{h}")
                for h in range(2)
            ]
        pv(pair, accs_by_unit[u], pts.pop(gk), kc)
        if kc == N_KC - 1:
            normalize(pair, qh, accs_by_unit[u])

    for gk in range(len(UNITS) * N_KC):
        u, kc = divmod(gk, N_KC)
        pair, qh = UNITS[u]
        pt = ptp.tile([128, 2, QH], BF16, tag="pt")
        pts[gk] = pt
        qk_exp(pair, kc, qh * QH, pt)
        if u == 0:
            # back-loaded weave: the PE clock is still ramping early in the
            # stream, so keep the first chunks light and catch up later
            _consume(vgen, (0, 2, 5)[(kc >= 3) + (kc >= 8)])
        if gk >= DELAY:
            emit_pv(gk - DELAY)
    _consume(vgen, 999)

    # ---------- drain: last unit's PV + first half of Wo ----------
    # Wo chunks 0-5 interleave with the PV sweep; 6-7 land right after the
    # final emit_pv so the PE stays busy while the last normalize chain
    # (DVE evict -> DMA bounce -> recip -> mult) runs.
    first = len(UNITS) * N_KC - DELAY
    for j in range(N_KC // 2):
        emit_pv(first + 2 * j)
        emit_pv(first + 2 * j + 1)
        if j < 6:
            ps = stp.tile([128, 1024], F32, tag="st", name="wops")
            wo_chunk(j, ps)
    for st_i in (6, 7):
        ps = stp.tile([128, 1024], F32, tag="st", name="wops")
        wo_chunk(st_i, ps)
    for st_i in range(N_KC // 2, S // 128):
        if st_i % 2 == 0:
            ps = stp.tile([128, 1024], F32, tag="st", name="wops")
        else:
            ps = accp.tile([128, 1024], F32, tag="acc", name="wops")
        wo_chunk(st_i, ps)

    phase_ctx.close()


def make_core_inputs(x, Wq, bq, Wk, bk, Wv, bv, Wo, bo, rel_bias):
    """Host-side shard prep. Returns list of 8 in_maps."""
    BF = ml_dtypes.bfloat16
    x = np.asarray(x, np.float32)
    in_maps = []
    WqT = np.ascontiguousarray(np.asarray(Wq, np.float32).T.astype(BF))
    WkT = np.ascontiguousarray(np.asarray(Wk, np.float32).T.astype(BF))
    WvT = np.ascontiguousarray(np.asarray(Wv, np.float32).T.astype(BF))
    WoT = np.ascontiguousarray(np.asarray(Wo, np.float32).T.astype(BF))
    rel = np.asarray(rel_bias, np.float32)
    xt = [np.ascontiguousarray(x[b].T.astype(BF)) for b in range(B)]

    # band multiplier: [p, h_local, m] = exp(bias(q,k) - c_past), q-k = m-128-p
    p_i = np.arange(128)[:, None]
    m_i = np.arange(BAND)[None, :]
    delta = np.clip(m_i - 128 - p_i, -MAX_REL, MAX_REL) + MAX_REL  # [128, 384]

    for c in range(N_CORES):
        b = c // CORES_PER_BATCH
        g = c % CORES_PER_BATCH
        c0 = g * CL
        heads = np.arange(g * HEADS_PER_CORE, (g + 1) * HEADS_PER_CORE)

        bqk = np.empty((128, 4), np.float32)
        bqk[:, 0] = np.asarray(bq, np.float32)[c0:c0 + 128]
        bqk[:, 1] = np.asarray(bq, np.float32)[c0 + 128:c0 + 256]
        bqk[:, 2] = np.asarray(bk, np.float32)[c0:c0 + 128]
        bqk[:, 3] = np.asarray(bk, np.float32)[c0 + 128:c0 + 256]

        band = np.empty((128, HEADS_PER_CORE, BAND), np.float32)
        abias = np.empty((128, 2 * HEADS_PER_CORE), np.float32)
        for i, hg in enumerate(heads):
            c_past = rel[hg, 2 * MAX_REL]
            band[:, i, :] = np.exp(rel[hg][delta] - c_past)
            abias[:, 2 * i] = np.exp(rel[hg, 0] - c_past)  # future multiplier
            abias[:, 2 * i + 1] = c_past
        in_maps.append({
            "xt": xt[b],
            "wqt": np.ascontiguousarray(WqT[:, c0:c0 + CL]),
            "wkt": np.ascontiguousarray(WkT[:, c0:c0 + CL]),
            "wvt": np.ascontiguousarray(WvT[:, c0:c0 + CL]),
            "wot": np.ascontiguousarray(WoT[c0:c0 + CL, :]),
            "bqk": bqk,
            "band": band.astype(BF),
            "abias": abias,
        })
    return in_maps


_NC_CACHE = {}


def get_program(**kw):
    key = tuple(sorted(kw.items()))
    if key not in _NC_CACHE:
        _NC_CACHE[key] = build_program(**kw)
    return _NC_CACHE[key]


def kernel(x, Wq, bq, Wk, bk, Wv, bv, Wo, bo, rel_bias):
    from concourse.bass_utils import run_bass_kernel_spmd

    nc = get_program()
    in_maps = make_core_inputs(x, Wq, bq, Wk, bk, Wv, bv, Wo, bo, rel_bias)
    res = run_bass_kernel_spmd(nc, in_maps, core_ids=list(range(N_CORES)))
    results = res.results

    Wo_np = np.asarray(Wo, np.float32)
    const = np.asarray(bv, np.float32) @ Wo_np.T + np.asarray(bo, np.float32)
    out = np.zeros((B, S, D), np.float32)
    for c in range(N_CORES):
        out[c // CORES_PER_BATCH] += results[c]["out_p"].astype(np.float32)
    out += const[None, None, :]
    return out
